# revision 11
# baseline (speedup 1.0000x reference)
"""Cox partial-likelihood loss on 8 Trainium2 NeuronCores.

reference:
    theta = hazard_pred.reshape(-1)                 # [n]
    R[i, j] = survtime[j] >= survtime[i]            # risk-set mask
    risk_sum[i] = sum_j exp(theta[j]) * R[i, j]
    loss = -mean((theta - log(risk_sum)) * censor)

Sharding: rows i are split across 8 cores (1024 rows each). Each core
computes its [8192 x 1024] slice of the risk mask in 64 chunks of 128
j's and contracts each chunk against exp(theta) on the TensorEngine,
accumulating risk_sum for its rows in PSUM. Mask generation is split
across three engines (chunk pattern c % 4):

  - DVE   (c%4 in {0,2}): tensor_scalar (s_i <= s_j)*2 -> {0,2} fp8
  - GPSIMD (c%4 == 1):    same op                      -> {0,2} fp8
  - ACT   (c%4 == 3):     Sign(s_j - s_i)              -> {-1,0,1} bf16

fp8 chunks run as DoubleRow matmuls at 2x PE rate: the stationary
operand packs e as an exact-split pair (a = f8(e), b = f8(e - a)) in
the two K-subrows, and the moving mask is read twice through a
0-stride access pattern, so each chunk contributes mask * (a + b)
with ~2^-8 relative weight error (bf16-class). ACT chunks use plain
bf16 matmuls with e16 = bf16(e).

Sign-encoding corrections (A = ACT chunk set): PSUM holds
    P[i] = sum_{D,G} 2*R_c[i] + sum_A (2*R_c[i] - S_c - tie_i)
so  risk_sum[i] = 0.5*(P[i] + e16[g_i]*w[i]) + 0.5*S_A,
with w[i] = 1 iff (i mod 64) in A (row i's self-tie chunk, sign(0)=0).
The e16*w row is added into PSUM via a K=1 bf16 matmul; the 0.5 scale
and S_A bias fold into the Ln activation. Exact non-diagonal survtime
ties inside A-chunks are the only unmodeled effect (~4 expected pairs,
each off by 0.5*e_j; ~1e-8 relative on the loss).

Host sums the 8 partial row-sums and applies -1/n.

j-index mapping: j = p*64 + c (p = SBUF partition, c = chunk column),
so survtime/theta load as contiguous [128, 64] tiles and chunk c uses
column c for both the per-partition compare scalar and the matmul
stationary operand.
"""

from contextlib import ExitStack, nullcontext

import numpy as np

import concourse.bacc as bacc
import concourse.bass as bass
import concourse.tile as tile
from concourse import mybir
from concourse.bass_utils import run_bass_kernel_spmd

DT = mybir.dt
AF = mybir.ActivationFunctionType
N = 8192
CORES = 8
NL = N // CORES       # 1024 local rows per core
NCHUNK = 64           # j-chunks of 128
NHALF = NL // 2       # matmul free-dim limit is 512


def _chunk_engine(c: int) -> str:
    return {0: "dve", 1: "gp", 2: "dve", 3: "act"}[c % 4]


def _dbl(ap):
    """Read a [128, F] AP twice as [128, 2, F] via a 0-stride middle dim."""
    return bass.AP(tensor=ap.tensor, offset=ap.offset,
                   ap=[ap.ap[0], [0, 2], ap.ap[1]])


_CACHE: dict = {}


def _emit_body(nc, const, masks, psums, tailp, st_all, th_all, st_loc, th_loc,
               cen_loc, wv, wc, partial):
    # j-major tiles: [p, c] holds index j = p*64 + c
    st_sb = const.tile([128, NCHUNK], DT.float32)
    nc.sync.dma_start(out=st_sb, in_=st_all[:].rearrange("(p c) -> p c", c=NCHUNK))
    th_sb = const.tile([128, NCHUNK], DT.float32)
    nc.sync.dma_start(out=th_sb, in_=th_all[:].rearrange("(p c) -> p c", c=NCHUNK))
    wc_sb = const.tile([128, NCHUNK], DT.float32)
    nc.sync.dma_start(out=wc_sb, in_=wc[:].rearrange("(p c) -> p c", c=NCHUNK))

    e32 = const.tile([128, NCHUNK], DT.float32)
    nc.scalar.activation(out=e32, in_=th_sb, func=AF.Exp)
    e16 = const.tile([128, NCHUNK], DT.bfloat16)
    nc.vector.tensor_copy(out=e16, in_=e32)

    # fp8 exact-split pair of e: ab8[:,0,:] = f8(e), ab8[:,1,:] = f8(e - f8(e))
    ab8 = const.tile([128, 2, NCHUNK], DT.float8e4)
    nc.vector.tensor_copy(out=ab8[:, 0, :], in_=e32)
    a32 = const.tile([128, NCHUNK], DT.float32)
    nc.vector.tensor_copy(out=a32, in_=ab8[:, 0, :])
    d32 = const.tile([128, NCHUNK], DT.float32)
    nc.vector.tensor_sub(d32, e32, a32)
    nc.vector.tensor_copy(out=ab8[:, 1, :], in_=d32)

    # tail inputs (loaded early; el32's Exp shares the ACT table with e32's)
    thl = tailp.tile([1, NL], DT.float32)
    nc.sync.dma_start(out=thl, in_=th_loc[:].rearrange("(o n) -> o n", o=1))
    cenl = tailp.tile([1, NL], DT.float32)
    nc.sync.dma_start(out=cenl, in_=cen_loc[:].rearrange("(o n) -> o n", o=1))
    wvl = tailp.tile([1, NL], DT.float32)
    nc.sync.dma_start(out=wvl, in_=wv[:].rearrange("(o n) -> o n", o=1))
    el32 = tailp.tile([1, NL], DT.float32)
    nc.scalar.activation(out=el32, in_=thl, func=AF.Exp)

    # S_A = sum of e16 over ACT-chunk columns (exact: reduce the bf16 values)
    ef = const.tile([128, NCHUNK], DT.float32)
    nc.vector.tensor_copy(out=ef, in_=e16)
    em = const.tile([128, NCHUNK], DT.float32)
    nc.vector.tensor_mul(em, ef, wc_sb)
    colsum = const.tile([128, 1], DT.float32)
    nc.vector.tensor_reduce(
        out=colsum, in_=em, axis=mybir.AxisListType.X, op=mybir.AluOpType.add
    )
    ones32 = const.tile([128, 1], DT.float32)
    nc.vector.memset(ones32, 1.0)
    psa = psums.tile([1, 1], DT.float32, tag="psa")
    nc.tensor.matmul(psa, ones32, colsum, start=True, stop=True)
    half_sa = const.tile([1, 1], DT.float32)
    nc.scalar.activation(out=half_sa, in_=psa, func=AF.Copy, scale=0.5)

    # diagonal-tie correction row: corr16 = bf16(e_local) * w  (exactly e16)
    corr32 = tailp.tile([1, NL], DT.float32)
    nc.gpsimd.tensor_mul(corr32, el32, wvl)
    corr16 = tailp.tile([1, NL], DT.bfloat16)
    nc.gpsimd.tensor_copy(out=corr16, in_=corr32)
    ones16 = const.tile([1, 1], DT.bfloat16)
    nc.vector.memset(ones16, 1.0)

    # local survtime broadcast to all partitions (free dim = local row i)
    si_b = const.tile([128, NL], DT.float32)
    st_loc_row = st_loc[:].rearrange("(o n) -> o n", o=1)
    nc.gpsimd.dma_start(out=si_b, in_=st_loc_row.partition_broadcast(128))

    # main loop: P[i] accumulates the encoded mask @ e contraction
    p0 = psums.tile([1, NHALF], DT.float32, tag="p0")
    p1 = psums.tile([1, NHALF], DT.float32, tag="p1")
    for c in range(NCHUNK):
        eng = _chunk_engine(c)
        if eng == "act":
            m = masks.tile([128, NL], DT.bfloat16, tag="ma")
            nc.scalar.activation(
                out=m, in_=si_b, func=AF.Sign, bias=st_sb[:, c : c + 1], scale=-1.0
            )
            nc.tensor.matmul(
                p0, e16[:, c : c + 1], m[:, 0:NHALF], start=False, stop=False
            )
            nc.tensor.matmul(
                p1, e16[:, c : c + 1], m[:, NHALF:NL], start=False, stop=False
            )
        else:
            m = masks.tile([128, NL], DT.float8e4, tag="m" + eng)
            ts = nc.vector if eng == "dve" else nc.gpsimd
            ts.tensor_scalar(
                out=m,
                in0=si_b,
                scalar1=st_sb[:, c : c + 1],
                scalar2=2.0,
                op0=mybir.AluOpType.is_le,
                op1=mybir.AluOpType.mult,
            )
            nc.tensor.matmul(
                p0, ab8[:, :, c : c + 1], _dbl(m[:, 0:NHALF]),
                start=(c == 0), stop=False,
                perf_mode=mybir.MatmulPerfMode.DoubleRow,
            )
            nc.tensor.matmul(
                p1, ab8[:, :, c : c + 1], _dbl(m[:, NHALF:NL]),
                start=(c == 0), stop=False,
                perf_mode=mybir.MatmulPerfMode.DoubleRow,
            )
    # fold the diagonal correction into PSUM (K=1 matmul), closing the group
    nc.tensor.matmul(p0, ones16, corr16[:, 0:NHALF], start=False, stop=True)
    nc.tensor.matmul(p1, ones16, corr16[:, NHALF:NL], start=False, stop=True)

    # tail: risk = 0.5*P + 0.5*S_A ; partial = sum((theta - ln(risk))*censor)
    lnt = tailp.tile([1, NL], DT.float32)
    nc.scalar.activation(out=lnt[:, 0:NHALF], in_=p0, func=AF.Ln,
                         bias=half_sa, scale=0.5)
    nc.scalar.activation(out=lnt[:, NHALF:NL], in_=p1, func=AF.Ln,
                         bias=half_sa, scale=0.5)
    d = tailp.tile([1, NL], DT.float32)
    nc.gpsimd.tensor_sub(d, thl, lnt)
    dc = tailp.tile([1, NL], DT.float32)
    nc.gpsimd.tensor_mul(dc, d, cenl)
    res = tailp.tile([1, 1], DT.float32)
    nc.vector.tensor_reduce(
        out=res, in_=dc, axis=mybir.AxisListType.X, op=mybir.AluOpType.add
    )
    nc.sync.dma_start(out=partial[:].rearrange("(o n) -> o n", o=1), in_=res)


def _build_nc(reps: int | None = None) -> bass.Bass:
    nc = bacc.Bacc()
    st_all = nc.declare_dram_parameter("st_all", [N], DT.float32, isOutput=False)
    th_all = nc.declare_dram_parameter("th_all", [N], DT.float32, isOutput=False)
    st_loc = nc.declare_dram_parameter("st_loc", [NL], DT.float32, isOutput=False)
    th_loc = nc.declare_dram_parameter("th_loc", [NL], DT.float32, isOutput=False)
    cen_loc = nc.declare_dram_parameter("cen_loc", [NL], DT.float32, isOutput=False)
    wv = nc.declare_dram_parameter("wv", [NL], DT.float32, isOutput=False)
    wc = nc.declare_dram_parameter("wc", [N], DT.float32, isOutput=False)
    partial = nc.declare_dram_parameter("partial", [1], DT.float32, isOutput=True)

    with tile.TileContext(nc) as tc, ExitStack() as ctx:
        const = ctx.enter_context(tc.tile_pool(name="const", bufs=1))
        masks = ctx.enter_context(tc.tile_pool(name="masks", bufs=5))
        psums = ctx.enter_context(tc.tile_pool(name="psums", bufs=1, space="PSUM"))
        tailp = ctx.enter_context(tc.tile_pool(name="tailp", bufs=1))

        loop = tc.For_i(0, reps, 1) if reps is not None else nullcontext()
        with loop:
            _emit_body(nc, const, masks, psums, tailp, st_all, th_all, st_loc,
                       th_loc, cen_loc, wv, wc, partial)

    nc.compile()
    return nc


def _get_nc() -> bass.Bass:
    if "nc" not in _CACHE:
        _CACHE["nc"] = _build_nc()
    return _CACHE["nc"]


def _w_patterns():
    cs = np.arange(NCHUNK)
    act = np.array([_chunk_engine(c) == "act" for c in cs], dtype=np.float32)
    wv = np.tile(act, NL // NCHUNK).astype(np.float32)   # w[i] = act[i % 64]
    wc = np.tile(act, N // NCHUNK).astype(np.float32)    # wc[j] = act[j % 64]
    return wv, wc


def make_in_maps(survtime: np.ndarray, theta: np.ndarray, censor: np.ndarray):
    st = np.ascontiguousarray(survtime, dtype=np.float32)
    th = np.ascontiguousarray(theta, dtype=np.float32).reshape(-1)
    cen = np.ascontiguousarray(censor, dtype=np.float32)
    wv, wc = _w_patterns()
    in_maps = []
    for k in range(CORES):
        lo, hi = k * NL, (k + 1) * NL
        in_maps.append(
            {
                "st_all": st,
                "th_all": th,
                "st_loc": st[lo:hi].copy(),
                "th_loc": th[lo:hi].copy(),
                "cen_loc": cen[lo:hi].copy(),
                "wv": wv,
                "wc": wc,
            }
        )
    return in_maps


def kernel(hazard_pred: np.ndarray, survtime: np.ndarray, censor: np.ndarray):
    nc = _get_nc()
    in_maps = make_in_maps(survtime, hazard_pred, censor)
    out = run_bass_kernel_spmd(nc, in_maps, list(range(CORES)))
    partials = np.array(
        [np.asarray(out.results[k]["partial"]).reshape(-1)[0] for k in range(CORES)],
        dtype=np.float64,
    )
    return np.float32(-partials.sum() / N)


# revision 26
# speedup vs baseline: 5.8847x; 5.8847x over previous
"""Cox partial-likelihood loss on 8 Trainium2 NeuronCores.

reference:
    theta = hazard_pred.reshape(-1)                 # [n]
    R[i, j] = survtime[j] >= survtime[i]            # risk-set mask
    risk_sum[i] = sum_j exp(theta[j]) * R[i, j]
    loss = -mean((theta - log(risk_sum)) * censor)

Sharding: rows i are split across 8 cores (1024 rows each). Each core
computes its [8192 x 1024] slice of the risk mask in 64 chunks of 128
j's and contracts each chunk against exp(theta) on the TensorEngine,
accumulating risk_sum for its rows in PSUM. Mask generation is split
across three engines (chunk pattern c % 4):

  - DVE   (c%4 in {0,2}): tensor_scalar (s_i <= s_j)*2 -> {0,2} fp8
  - GPSIMD (c%4 == 1):    same op                      -> {0,2} fp8
  - ACT   (c%4 == 3):     Sign(s_j - s_i)              -> {-1,0,1} bf16

fp8 chunks run as DoubleRow matmuls at 2x PE rate: the stationary
operand packs e as an exact-split pair (a = f8(e), b = f8(e - a)) in
the two K-subrows, and the moving mask is read twice through a
0-stride access pattern, so each chunk contributes mask * (a + b)
with ~2^-8 relative weight error (bf16-class). ACT chunks use plain
bf16 matmuls with e16 = bf16(e).

Sign-encoding corrections (A = ACT chunk set): PSUM holds
    P[i] = sum_{D,G} 2*R_c[i] + sum_A (2*R_c[i] - S_c - tie_i)
so  risk_sum[i] = 0.5*(P[i] + e16[g_i]*w[i]) + 0.5*S_A,
with w[i] = 1 iff (i mod 64) in A (row i's self-tie chunk, sign(0)=0).
The e16*w row is added into PSUM via a K=1 bf16 matmul; the 0.5 scale
and S_A bias fold into the Ln activation. Exact non-diagonal survtime
ties inside A-chunks are the only unmodeled effect (~4 expected pairs,
each off by 0.5*e_j; ~1e-8 relative on the loss).

Host sums the 8 partial row-sums and applies -1/n.

j-index mapping: j = p*64 + c (p = SBUF partition, c = chunk column),
so survtime/theta load as contiguous [128, 64] tiles and chunk c uses
column c for both the per-partition compare scalar and the matmul
stationary operand.
"""

from contextlib import ExitStack, nullcontext

import numpy as np

import concourse.bacc as bacc
import concourse.bass as bass
import concourse.tile as tile
from concourse import mybir
from concourse.bass_utils import run_bass_kernel_spmd

DT = mybir.dt
AF = mybir.ActivationFunctionType
N = 8192
CORES = 8
NL = N // CORES       # 1024 local rows per core
NCHUNK = 64           # j-chunks of 128
NHALF = NL // 2       # matmul free-dim limit is 512


# 40 DVE : 24 ACT chunk split (measured-balanced on HW: DVE ~684ns/chunk,
# ACT ~1163ns/chunk, PE consumes at ~412ns/chunk)
PATTERN = {0: "dve", 1: "dve", 2: "dve", 3: "act",
           4: "dve", 5: "act", 6: "dve", 7: "act"}
USE_FP8 = False  # fp8+DoubleRow measured slower than bf16 on HW; keep bf16
MASK_BUFS = 4    # buffers per mask tag
SIB_MODE = "hw4"  # 4-way HWDGE split broadcast (frees Pool, shortens head)
TAIL_GP = False  # gpsimd elementwise is slow on HW; tail on DVE
PAIRED = False   # pair-grained tiles showed no gain (region-based deps)


def _chunk_engine(c: int) -> str:
    return PATTERN[c % len(PATTERN)]


def _dbl(ap):
    """Read a [128, F] AP twice as [128, 2, F] via a 0-stride middle dim."""
    return bass.AP(tensor=ap.tensor, offset=ap.offset,
                   ap=[ap.ap[0], [0, 2], ap.ap[1]])


_CACHE: dict = {}


def _emit_body(nc, const, masks, psums, tailp, st_all, th_all, st_loc, th_loc,
               cen_loc, wv, wc, partial):
    # j-major tiles: [p, c] holds index j = p*64 + c
    st_sb = const.tile([128, NCHUNK], DT.float32)
    nc.sync.dma_start(out=st_sb, in_=st_all[:].rearrange("(p c) -> p c", c=NCHUNK))
    th_sb = const.tile([128, NCHUNK], DT.float32)
    nc.sync.dma_start(out=th_sb, in_=th_all[:].rearrange("(p c) -> p c", c=NCHUNK))
    wc_sb = const.tile([128, NCHUNK], DT.float32)
    nc.sync.dma_start(out=wc_sb, in_=wc[:].rearrange("(p c) -> p c", c=NCHUNK))

    e32 = const.tile([128, NCHUNK], DT.float32)
    nc.scalar.activation(out=e32, in_=th_sb, func=AF.Exp)
    e16 = const.tile([128, NCHUNK], DT.bfloat16)
    nc.vector.tensor_copy(out=e16, in_=e32)

    # fp8 exact-split pair of e: ab8[:,0,:] = f8(e), ab8[:,1,:] = f8(e - f8(e))
    ab8 = None
    if True:
        ab8 = const.tile([128, 2, NCHUNK], DT.float8e4)
        nc.vector.tensor_copy(out=ab8[:, 0, :], in_=e32)
        a32 = const.tile([128, NCHUNK], DT.float32)
        nc.vector.tensor_copy(out=a32, in_=ab8[:, 0, :])
        d32 = const.tile([128, NCHUNK], DT.float32)
        nc.vector.tensor_sub(d32, e32, a32)
        nc.vector.tensor_copy(out=ab8[:, 1, :], in_=d32)

    # tail inputs (loaded early; el32's Exp shares the ACT table with e32's)
    thl = tailp.tile([1, NL], DT.float32)
    nc.sync.dma_start(out=thl, in_=th_loc[:].rearrange("(o n) -> o n", o=1))
    cenl = tailp.tile([1, NL], DT.float32)
    nc.sync.dma_start(out=cenl, in_=cen_loc[:].rearrange("(o n) -> o n", o=1))
    wvl = tailp.tile([1, NL], DT.float32)
    nc.sync.dma_start(out=wvl, in_=wv[:].rearrange("(o n) -> o n", o=1))
    el32 = tailp.tile([1, NL], DT.float32)
    nc.scalar.activation(out=el32, in_=thl, func=AF.Exp)

    # S_A = sum of e16 over ACT-chunk columns (exact: reduce the bf16 values)
    ef = const.tile([128, NCHUNK], DT.float32)
    nc.vector.tensor_copy(out=ef, in_=e16)
    em = const.tile([128, NCHUNK], DT.float32)
    nc.vector.tensor_mul(em, ef, wc_sb)
    colsum = const.tile([128, 1], DT.float32)
    nc.vector.tensor_reduce(
        out=colsum, in_=em, axis=mybir.AxisListType.X, op=mybir.AluOpType.add
    )
    ones32 = const.tile([128, 1], DT.float32)
    nc.vector.memset(ones32, 1.0)
    psa = psums.tile([1, 1], DT.float32, tag="psa")
    nc.tensor.matmul(psa, ones32, colsum, start=True, stop=True)
    half_sa = const.tile([1, 1], DT.float32)
    nc.scalar.activation(out=half_sa, in_=psa, func=AF.Copy, scale=0.5)

    # diagonal-tie correction row: corr16 = bf16(e_local) * w  (exactly e16)
    corr32 = tailp.tile([1, NL], DT.float32)
    nc.gpsimd.tensor_mul(corr32, el32, wvl)
    corr16 = tailp.tile([1, NL], DT.bfloat16)
    nc.gpsimd.tensor_copy(out=corr16, in_=corr32)
    ones16 = const.tile([1, 1], DT.bfloat16)
    nc.vector.memset(ones16, 1.0)

    # local survtime broadcast to all partitions (free dim = local row i)
    si_b = const.tile([128, NL], DT.float32)
    st_loc_row = st_loc[:].rearrange("(o n) -> o n", o=1)
    if SIB_MODE == "gp":
        nc.gpsimd.dma_start(out=si_b, in_=st_loc_row.partition_broadcast(128))
    else:
        for q in range(4):
            nc.sync.dma_start(
                out=si_b[q * 32 : (q + 1) * 32, :],
                in_=st_loc_row.partition_broadcast(32),
            )

    # main loop: P[i] accumulates the encoded mask @ e contraction
    p0 = psums.tile([1, NHALF], DT.float32, tag="p0")
    p1 = psums.tile([1, NHALF], DT.float32, tag="p1")
    if PAIRED:
        # two same-engine chunks share one mask tile: one producer->PE
        # handshake per pair instead of per chunk
        for t in range(NCHUNK // 2):
            c0 = 2 * t
            eng = _chunk_engine(c0)
            assert _chunk_engine(c0 + 1) == eng, "PAIRED needs aligned pattern"
            if eng == "act":
                m = masks.tile([128, 2, NL], DT.bfloat16, tag="ma")
                for g in (0, 1):
                    nc.scalar.activation(
                        out=m[:, g, :], in_=si_b, func=AF.Sign,
                        bias=st_sb[:, c0 + g : c0 + g + 1], scale=-1.0,
                    )
            else:
                m = masks.tile([128, 2, NL], DT.bfloat16, tag="md")
                for g in (0, 1):
                    nc.vector.tensor_scalar(
                        out=m[:, g, :],
                        in0=si_b,
                        scalar1=st_sb[:, c0 + g : c0 + g + 1],
                        scalar2=2.0,
                        op0=mybir.AluOpType.is_le,
                        op1=mybir.AluOpType.mult,
                    )
            for g in (0, 1):
                nc.tensor.matmul(
                    p0, e16[:, c0 + g : c0 + g + 1], m[:, g, 0:NHALF],
                    start=(c0 + g == 0), stop=False,
                )
                nc.tensor.matmul(
                    p1, e16[:, c0 + g : c0 + g + 1], m[:, g, NHALF:NL],
                    start=(c0 + g == 0), stop=False,
                )
        emit_chunks = []
    else:
        emit_chunks = list(range(NCHUNK))
    for c in emit_chunks:
        eng = _chunk_engine(c)
        if eng == "act":
            m = masks.tile([128, NL], DT.bfloat16, tag="ma")
            nc.scalar.activation(
                out=m, in_=si_b, func=AF.Sign, bias=st_sb[:, c : c + 1], scale=-1.0
            )
            nc.tensor.matmul(
                p0, e16[:, c : c + 1], m[:, 0:NHALF], start=False, stop=False
            )
            nc.tensor.matmul(
                p1, e16[:, c : c + 1], m[:, NHALF:NL], start=False, stop=False
            )
        else:
            mdt = DT.float8e4 if USE_FP8 else DT.bfloat16
            m = masks.tile([128, NL], mdt, tag="m" + eng)
            ts = nc.vector if eng == "dve" else nc.gpsimd
            ts.tensor_scalar(
                out=m,
                in0=si_b,
                scalar1=st_sb[:, c : c + 1],
                scalar2=2.0,
                op0=mybir.AluOpType.is_le,
                op1=mybir.AluOpType.mult,
            )
            if USE_FP8:
                nc.tensor.matmul(
                    p0, ab8[:, :, c : c + 1], _dbl(m[:, 0:NHALF]),
                    start=(c == 0), stop=False,
                    perf_mode=mybir.MatmulPerfMode.DoubleRow,
                )
                nc.tensor.matmul(
                    p1, ab8[:, :, c : c + 1], _dbl(m[:, NHALF:NL]),
                    start=(c == 0), stop=False,
                    perf_mode=mybir.MatmulPerfMode.DoubleRow,
                )
            else:
                nc.tensor.matmul(
                    p0, e16[:, c : c + 1], m[:, 0:NHALF],
                    start=(c == 0), stop=False,
                )
                nc.tensor.matmul(
                    p1, e16[:, c : c + 1], m[:, NHALF:NL],
                    start=(c == 0), stop=False,
                )
    # fold the diagonal correction into PSUM (K=1 matmul), closing the group
    nc.tensor.matmul(p0, ones16, corr16[:, 0:NHALF], start=False, stop=True)
    nc.tensor.matmul(p1, ones16, corr16[:, NHALF:NL], start=False, stop=True)

    # tail: risk = 0.5*P + 0.5*S_A ; partial = sum((theta - ln(risk))*censor)
    lnt = tailp.tile([1, NL], DT.float32)
    nc.scalar.activation(out=lnt[:, 0:NHALF], in_=p0, func=AF.Ln,
                         bias=half_sa, scale=0.5)
    nc.scalar.activation(out=lnt[:, NHALF:NL], in_=p1, func=AF.Ln,
                         bias=half_sa, scale=0.5)
    tail_eng = nc.gpsimd if TAIL_GP else nc.vector
    d = tailp.tile([1, NL], DT.float32)
    tail_eng.tensor_sub(d, thl, lnt)
    dc = tailp.tile([1, NL], DT.float32)
    tail_eng.tensor_mul(dc, d, cenl)
    res = tailp.tile([1, 1], DT.float32)
    nc.vector.tensor_reduce(
        out=res, in_=dc, axis=mybir.AxisListType.X, op=mybir.AluOpType.add
    )
    nc.sync.dma_start(out=partial[:].rearrange("(o n) -> o n", o=1), in_=res)


def _build_nc(reps: int | None = None) -> bass.Bass:
    nc = bacc.Bacc()
    st_all = nc.declare_dram_parameter("st_all", [N], DT.float32, isOutput=False)
    th_all = nc.declare_dram_parameter("th_all", [N], DT.float32, isOutput=False)
    st_loc = nc.declare_dram_parameter("st_loc", [NL], DT.float32, isOutput=False)
    th_loc = nc.declare_dram_parameter("th_loc", [NL], DT.float32, isOutput=False)
    cen_loc = nc.declare_dram_parameter("cen_loc", [NL], DT.float32, isOutput=False)
    wv = nc.declare_dram_parameter("wv", [NL], DT.float32, isOutput=False)
    wc = nc.declare_dram_parameter("wc", [N], DT.float32, isOutput=False)
    partial = nc.declare_dram_parameter("partial", [1], DT.float32, isOutput=True)

    with tile.TileContext(nc) as tc, ExitStack() as ctx:
        const = ctx.enter_context(tc.tile_pool(name="const", bufs=1))
        masks = ctx.enter_context(tc.tile_pool(name="masks", bufs=MASK_BUFS))
        psums = ctx.enter_context(tc.tile_pool(name="psums", bufs=1, space="PSUM"))
        tailp = ctx.enter_context(tc.tile_pool(name="tailp", bufs=1))

        loop = (
            tc.For_i(0, reps, 1,
                     hint_engines=(mybir.EngineType.PE, mybir.EngineType.DVE))
            if reps is not None
            else nullcontext()
        )
        with loop:
            _emit_body(nc, const, masks, psums, tailp, st_all, th_all, st_loc,
                       th_loc, cen_loc, wv, wc, partial)

    nc.compile()
    return nc


def _get_nc() -> bass.Bass:
    if "nc" not in _CACHE:
        _CACHE["nc"] = _build_nc()
    return _CACHE["nc"]


def _w_patterns():
    cs = np.arange(NCHUNK)
    act = np.array([_chunk_engine(c) == "act" for c in cs], dtype=np.float32)
    wv = np.tile(act, NL // NCHUNK).astype(np.float32)   # w[i] = act[i % 64]
    wc = np.tile(act, N // NCHUNK).astype(np.float32)    # wc[j] = act[j % 64]
    return wv, wc


def make_in_maps(survtime: np.ndarray, theta: np.ndarray, censor: np.ndarray):
    st = np.ascontiguousarray(survtime, dtype=np.float32)
    th = np.ascontiguousarray(theta, dtype=np.float32).reshape(-1)
    cen = np.ascontiguousarray(censor, dtype=np.float32)
    wv, wc = _w_patterns()
    in_maps = []
    for k in range(CORES):
        lo, hi = k * NL, (k + 1) * NL
        in_maps.append(
            {
                "st_all": st,
                "th_all": th,
                "st_loc": st[lo:hi].copy(),
                "th_loc": th[lo:hi].copy(),
                "cen_loc": cen[lo:hi].copy(),
                "wv": wv,
                "wc": wc,
            }
        )
    return in_maps


def kernel(hazard_pred: np.ndarray, survtime: np.ndarray, censor: np.ndarray):
    nc = _get_nc()
    in_maps = make_in_maps(survtime, hazard_pred, censor)
    out = run_bass_kernel_spmd(nc, in_maps, list(range(CORES)))
    partials = np.array(
        [np.asarray(out.results[k]["partial"]).reshape(-1)[0] for k in range(CORES)],
        dtype=np.float64,
    )
    return np.float32(-partials.sum() / N)
